# revision 57
# baseline (speedup 1.0000x reference)
"""Trainium2 Bass kernel for nn_Att_76381698392129.

kernel(**inputs) -> np.ndarray, self-contained.

Reference math:
    v     = x @ value_w.T                      [B, N, 3]
    score = (key_w @ query_w) / 16             [N, N]
    l1    = sum_o |score[i, o]|
    s_n   = score / max(l1, 1e-12)
    y     = einsum("io,bid->bod", s_n, v)      [B, N, 3]

Factored algorithm (never materializes the N x N score matrix off-chip):
    l1_q[i] = sum_o |(kw @ 16*qw)[i, o]|     (the only big matmul; fp8)
    r[i]    = 2^18 / max(l1_q[i], eps)
    T       = kw^T @ (V * r)    [H, B*3],  V[i,(b,d)] = (x @ vw^T)[b, i, d]
    y[b,o,e]= 16 * 2^-18 * (qw^T @ T)[o, (b,e)]   (scales folded on-chip)

Two 8-core SPMD launches (collectives measured 16-100us here, so the
[256,192] partial-T reduction goes through a host gather instead):
  Launch A - score rows sharded 8 ways: fp8 DoubleRow matmuls (K=256 per
    instruction) into a 6-bank PSUM ring; |.|-row-reduction split between
    ACT (Abs+accum_out) and DVE (tensor_reduce) as interleaved windowed
    spans emitted right after their last tile, so PE and both consumer
    engines pipeline; per-core partial T accumulated in PSUM banks 6-7.
  Launch B - y row shard per core from the host-summed T (bf16 matmuls).
"""

import os
from contextlib import ExitStack

import numpy as np

import concourse.bass as bass
import concourse.mybir as mybir
import concourse.tile as tile

F32 = mybir.dt.float32
BF16 = mybir.dt.bfloat16
FP8 = mybir.dt.float8e4
AX = mybir.AxisListType
ALU = mybir.AluOpType
ACTF = mybir.ActivationFunctionType
PM = mybir.MatmulPerfMode

N = 5023
H_DIM = 256
B = 64
BD = B * 3
N_CORES = 8
N_PAD = 5120
S = N_PAD // N_CORES      # 640 rows per core
MT = S // 128             # 5 row blocks
OT = N_PAD // 512         # 10 col chunks
RING = 6                  # PSUM banks in the score ring
ACT_W1 = 1664             # ACT cols in tiles [0,6) of each m
ACT_W2 = 1152             # ACT cols in tiles [6,10)
R_SCALE = 2.0 ** -18      # keeps vsc/T in comfortable range
T_SCALE = 16.0 * 2.0 ** -18  # folds the 16*qw and the R_SCALE back out
EPS = 1e-3                # only guards all-zero pad rows

LAST_HW_EXEC_NS = None
LAST_PHASE_A_NS = None
LAST_PHASE_B_NS = None

_PATCHED = False


def _patch_tile_drain():
    """This walrus build rejects >1 sync-wait on an InstDrain; re-emit the
    final drain's waits as individual wait_ge instructions."""
    global _PATCHED
    if _PATCHED:
        return
    _PATCHED = True
    import bass_rust

    def _drain_and_barrier(self, tick_clock, wait_clock):
        nc = self.nc
        probe = nc.sync.nop(nofuse=True, hint="drain_waits")
        wait_clock.add_sem_waits(
            probe.ins, bass_rust.ScopedClock({None: tick_clock.global_clock})
        )
        waits = list(probe.ins.sync_info.on_wait or []) if probe.ins.sync_info else []
        if probe.ins.sync_info is not None:
            probe.ins.sync_info.on_wait = []
        handles = {h.num: h for h in self.sems.allocated().values()}
        for w in waits:
            h = handles.get(w.id)
            assert h is not None, f"no handle for sem wait {w}"
            assert w.wait_mode == "sem-ge-imm", w
            nc.sync.wait_ge(h, w.wait_value)
        nc.sync.drain()
        nc.all_engine_barrier()
        popped = nc._tile_sem_poison_stack.pop()
        assert popped is self._sem_poison
        nc.clear_and_free_semaphores(list(self.sems.allocated().values()))
        nc.all_engine_barrier()

    tile.TileContext._drain_and_barrier = _drain_and_barrier


def _fix_multiwait(nc, max_waits=1):
    """This walrus build accepts at most one sync-wait command per
    instruction; peel extra waits onto same-engine nops just ahead."""
    f = nc.m.functions[0]
    all_blocks = list(f.blocks)
    for blk in all_blocks:
        insts = blk.instructions
        new = []
        for inst in insts:
            si = inst.sync_info
            w = list(si.on_wait) if si and si.on_wait else []
            if len(w) > max_waits:
                keep = w[-max_waits:]
                for extra in w[:-max_waits]:
                    nop = nc.engines[inst.engine].nop(
                        nofuse=True, hint="waitfix").ins
                    removed = False
                    for b2 in all_blocks:
                        l2 = b2.instructions
                        for k in range(len(l2) - 1, -1, -1):
                            if l2[k] is nop:
                                del l2[k]
                                removed = True
                                break
                        if removed:
                            break
                    assert removed, "waitfix nop not found in any block"
                    if nop.sync_info is None:
                        nop.sync_info = mybir.SyncInfo(on_wait=[extra],
                                                       on_update=[])
                    else:
                        nop.sync_info.on_wait = [extra]
                    new.append(nop)
                si.on_wait = keep
            new.append(inst)
        insts[:] = new
    return nc


# per-m PSUM ring groups: (n_tiles, [(lane, col_lo, col_hi), ...])
# lane "ap" = ACT Abs+accum_out; "dve" = DVE tensor_reduce from PSUM.
# ACT 2816 / DVE 2304 cols per m; 2-bank groups, pool bufs=3 so PE can
# run two groups ahead of the consumers.
GROUPS = [
    (2, [("ap", 0, 1024)]),
    (2, [("dve", 0, 1024)]),
    (2, [("ap", 0, 1024)]),
    (2, [("dve", 0, 1024)]),
    (2, [("ap", 0, 256), ("dve", 256, 1024)]),
]


def _build_phase_a():
    dbg = os.environ.get("ATT_DEBUG", "0") == "1"
    nc = bass.Bass("TRN2", target_bir_lowering=False, debug=False)
    KWT_W = MT * 2 * 128                      # 1280 fp8 cols of kwt prefix
    # fp8 image: [kwt | qw chunk 0..9]; the first DMA carries kwt+chunk0
    q8_d = nc.dram_tensor("q8", [128, KWT_W + OT * 2 * 512], FP8,
                          kind="ExternalInput")
    # bf16 image: [vv | kw]
    VV_W = MT * BD
    b16_d = nc.dram_tensor("b16", [128, VV_W + MT * H_DIM], BF16,
                           kind="ExternalInput")
    tv_d = nc.dram_tensor("tv", [128, 2 * BD], BF16, kind="ExternalOutput")
    if dbg:
        dl1_d = nc.dram_tensor("dl1", [128, MT], F32, kind="ExternalOutput")

    with tile.TileContext(nc) as tc, ExitStack() as ctx:
        sb = ctx.enter_context(tc.tile_pool(name="sb", bufs=1))
        scr = ctx.enter_context(tc.tile_pool(name="scr", bufs=2))
        ring_pool = ctx.enter_context(tc.tile_pool(name="ring", bufs=3, space="PSUM"))
        psT_pool = ctx.enter_context(tc.tile_pool(name="psT", bufs=1, space="PSUM"))

        # kwt(m0)+chunk0, kwt(m1-4), and chunks 1-9 are all separate tiles:
        # dependency tracking is tile-granular, so the first matmul waits
        # only on the 164KB it actually needs
        head_sb = sb.tile([128, 256 + 1024], FP8, name="head_sb", tag="head")
        kwtr_sb = sb.tile([128, KWT_W - 256], FP8, name="kwtr_sb", tag="kwtr")
        qw_c = [None] * OT
        for c in range(1, OT):
            qw_c[c] = sb.tile([128, 1024], FP8, name=f"qwc{c}", tag=f"qwc{c}")
        b16_sb = sb.tile([128, VV_W + MT * H_DIM], BF16,
                         name="b16_sb", tag="b16")

        # critical-path DMAs ride the ACT HWDGE ring, bulk alternates rings
        nc.scalar.dma_start(head_sb[:, 0:256], q8_d.ap()[:, 0:256])
        nc.scalar.dma_start(head_sb[:, 256:],
                            q8_d.ap()[:, KWT_W:KWT_W + 1024])
        nc.scalar.dma_start(kwtr_sb[:], q8_d.ap()[:, 256:KWT_W])
        for c in range(1, OT):
            eng = nc.scalar if c % 2 == 0 else nc.sync
            eng.dma_start(
                qw_c[c][:],
                q8_d.ap()[:, KWT_W + c * 1024:KWT_W + (c + 1) * 1024])
        nc.sync.dma_start(b16_sb[:], b16_d.ap())

        def kwt_ap(m):
            if m == 0:
                return head_sb[:, 0:256]
            return kwtr_sb[:, (m - 1) * 256:m * 256]

        def qw_ap(o):
            if o == 0:
                return head_sb[:, 256:256 + 1024]
            return qw_c[o][:]

        def kw_ap(m, h):
            base = VV_W + m * 256 + h * 128
            return b16_sb[:, base:base + 128]

        def vv_ap(m):
            return b16_sb[:, m * BD:(m + 1) * BD]

        # h0/h1 in separate banks: a second start=True in the same bank
        # wipes the first group's data (bank-granular zero region)
        psT = psT_pool.tile([128, 2 * 512], F32, name="psT", tag="psT")

        part = sb.tile([128, MT * 8], F32, name="part", tag="part")
        l1 = sb.tile([128, MT], F32, name="l1", tag="l1")
        rr = sb.tile([128, MT], F32, name="rr", tag="rr")

        pend_t = None  # deferred (m, vsc): T matmuls run one lap later

        def _emit_t(mm, vscm):
            for h in range(2):
                nc.tensor.matmul(
                    psT[:, h * 512:h * 512 + BD],
                    kw_ap(mm, h),
                    vscm[:],
                    start=(mm == 0), stop=(mm == MT - 1),
                )

        for m in range(MT):
            np_parts = 0
            o = 0
            for gi, (ntiles, lanes) in enumerate(GROUPS):
                grp = ring_pool.tile([128, 2 * 512], F32,
                                     name=f"g{m}_{gi}", tag="ring")
                for tj in range(ntiles):
                    nc.tensor.matmul(
                        grp[:, tj * 512:(tj + 1) * 512],
                        kwt_ap(m).rearrange("p (h i) -> p h i", h=2),
                        qw_ap(o).rearrange("p (h n) -> p h n", h=2),
                        start=True, stop=True, perf_mode=PM.DoubleRow,
                    )
                    o += 1
                    if o == 6 and pend_t is not None:
                        _emit_t(*pend_t)
                for lane, lo, hi in lanes:
                    dst = part[:, m * 8 + np_parts:m * 8 + np_parts + 1]
                    if lane == "ap":
                        sa = scr.tile([128, 1536], BF16, name="sa", tag="sa")
                        nc.scalar.activation(sa[:, 0:hi - lo], grp[:, lo:hi],
                                             ACTF.Abs, accum_out=dst)
                    else:
                        nc.vector.tensor_reduce(dst, grp[:, lo:hi],
                                                axis=AX.X, op=ALU.add,
                                                apply_absolute_value=True)
                    np_parts += 1

            # guard: ACT's accum_out lands in a READ_ACCUM micro-op after
            # the ABS; a zero column written afterwards (and included in
            # the combine) guarantees the partials are drained before the
            # combine reads them.
            nc.scalar.activation(part[:, m * 8 + np_parts:m * 8 + np_parts + 1],
                                 part[:, m * 8:m * 8 + 1],
                                 ACTF.Copy, scale=0.0)
            np_parts += 1
            # l1 -> r chain entirely on DVE (FIFO keeps it ordered)
            nc.vector.tensor_reduce(l1[:, m:m + 1],
                                    part[:, m * 8:m * 8 + np_parts],
                                    axis=AX.X, op=ALU.add)
            nc.vector.tensor_scalar(l1[:, m:m + 1], l1[:, m:m + 1],
                                    EPS, R_SCALE, op0=ALU.max, op1=ALU.mult)
            nc.vector.reciprocal(rr[:, m:m + 1], l1[:, m:m + 1])
            vsc = scr.tile([128, BD], BF16, name="vsc", tag="vsc")
            nc.vector.tensor_scalar_mul(vsc[:], vv_ap(m), rr[:, m:m + 1])
            pend_t = (m, vsc)

        # final T matmuls interleaved with per-half copy + DMA out
        # (h0's copy/DMA overlap h1's matmul); host sums the 8 partials
        mmf, vscf = pend_t
        t_sb = sb.tile([128, 2 * BD], BF16, name="t_sb", tag="t_sb")
        for h in range(2):
            nc.tensor.matmul(
                psT[:, h * 512:h * 512 + BD],
                kw_ap(mmf, h),
                vscf[:],
                start=(mmf == 0), stop=(mmf == MT - 1),
            )
            nc.scalar.activation(
                t_sb[:, h * BD:(h + 1) * BD],
                psT[:, h * 512:h * 512 + BD],
                ACTF.Copy, scale=T_SCALE)
            nc.sync.dma_start(tv_d.ap()[:, h * BD:(h + 1) * BD],
                              t_sb[:, h * BD:(h + 1) * BD])
        if dbg:
            dl1_sb = sb.tile([128, MT], F32, name="dl1_sb", tag="dl1_sb")
            nc.vector.tensor_copy(dl1_sb[:], l1[:])
            nc.sync.dma_start(dl1_d.ap(), dl1_sb[:])

    return _fix_multiwait(nc)


def _build_phase_b():
    nc = bass.Bass("TRN2", target_bir_lowering=False, debug=False)
    qwy_d = nc.dram_tensor("qwy", [128, MT * 2 * 128], BF16, kind="ExternalInput")
    ts_d = nc.dram_tensor("tsum", [128, 2 * BD], BF16, kind="ExternalInput")
    y_d = nc.dram_tensor("y", [128, MT * BD], BF16, kind="ExternalOutput")

    with tile.TileContext(nc) as tc, ExitStack() as ctx:
        sb = ctx.enter_context(tc.tile_pool(name="sb", bufs=1))
        ps_pool = ctx.enter_context(tc.tile_pool(name="ps", bufs=1, space="PSUM"))

        # ts rides the ACT HWDGE ring so the first matmul is not queued
        # behind the qwy chunks on the SP ring; qwy chunks are separate
        # tiles so each jb's matmuls wait only their own chunk
        ts_sb = sb.tile([128, 2 * BD], BF16, name="ts_sb", tag="ts")
        nc.scalar.dma_start(ts_sb[:], ts_d.ap())
        qwy_c = [sb.tile([128, 256], BF16, name=f"qwyc{jb}", tag=f"qwyc{jb}")
                 for jb in range(MT)]
        for jb in range(MT):
            nc.sync.dma_start(qwy_c[jb][:],
                              qwy_d.ap()[:, jb * 256:(jb + 1) * 256])

        # warm the PE during the DMA fill (y matmuls otherwise run at the
        # cold HAM clock)
        wrm_in = sb.tile([128, 512], BF16, name="wrm_in", tag="wrm_in")
        nc.vector.memset(wrm_in[0:1, :], 0.0)
        wrm_ps = ps_pool.tile([128, 512], F32, name="wrm_ps", tag="wps")
        for _ in range(6):
            nc.tensor.matmul(wrm_ps[:], wrm_in[:, 0:128], wrm_in[:],
                             start=True, stop=True)

        ps = ps_pool.tile([128, MT * 512], F32, name="ps", tag="ps")
        y_sb = sb.tile([128, MT * BD], BF16, name="y_sb", tag="y_sb")
        for jb in range(MT):
            ycol = jb * 512
            for h in range(2):
                nc.tensor.matmul(
                    ps[:, ycol:ycol + BD],
                    qwy_c[jb][:, h * 128:(h + 1) * 128],
                    ts_sb[:, h * BD:(h + 1) * BD],
                    start=(h == 0), stop=(h == 1),
                )
            dst = y_sb[:, jb * BD:(jb + 1) * BD]
            if jb % 2 == 0:
                nc.scalar.activation(dst, ps[:, ycol:ycol + BD], ACTF.Copy)
            else:
                nc.vector.tensor_copy(dst, ps[:, ycol:ycol + BD])
            nc.sync.dma_start(y_d.ap()[:, jb * BD:(jb + 1) * BD], dst)

    return _fix_multiwait(nc)


_NC_A = None
_NC_B = None


def _get_programs():
    global _NC_A, _NC_B
    if _NC_A is None:
        _patch_tile_drain()
        _NC_A = _build_phase_a()
        _NC_B = _build_phase_b()
    return _NC_A, _NC_B


def _quant_fp8(a):
    import ml_dtypes
    return np.clip(a, -240.0, 240.0).astype(ml_dtypes.float8_e4m3fn)


def _run_spmd(nc, in_maps, profile):
    if profile:
        from concourse.bass_utils import run_bass_kernel_spmd
        r = run_bass_kernel_spmd(nc, in_maps, core_ids=list(range(N_CORES)),
                                 trace=True, trace_cores=[0])
        return r.results, r.exec_time_ns
    from concourse import bass2jax
    return bass2jax.run_bass_via_pjrt(nc, in_maps, n_cores=N_CORES), None


def kernel(x, key_w, query_w, value_w):
    global LAST_HW_EXEC_NS, LAST_PHASE_A_NS, LAST_PHASE_B_NS
    import ml_dtypes

    x = np.asarray(x, dtype=np.float32)
    key_w = np.asarray(key_w, dtype=np.float32)
    query_w = np.asarray(query_w, dtype=np.float32)
    value_w = np.asarray(value_w, dtype=np.float32)

    profile = os.environ.get("ATT_PROFILE", "0") == "1"
    nc_a, nc_b = _get_programs()

    # ---- host-side layout prep ----
    kw_pad = np.zeros((N_PAD, H_DIM), np.float32)
    kw_pad[:N] = key_w
    qw_pad = np.zeros((H_DIM, N_PAD), np.float32)
    qw_pad[:, :N] = query_w
    # v = x @ vw^T, laid out [N_pad, B*3]
    v = np.einsum("bnd,ed->bne", x, value_w)
    v_pad = np.zeros((N_PAD, BD), np.float32)
    v_pad[:N] = np.ascontiguousarray(v.transpose(1, 0, 2)).reshape(N, BD)

    kw8 = _quant_fp8(kw_pad)         # [N_PAD, H]
    qw8 = _quant_fp8(16.0 * qw_pad)  # [H, N_PAD]
    kwb = kw_pad.astype(ml_dtypes.bfloat16)
    qwb = qw_pad.astype(ml_dtypes.bfloat16)
    vvb = v_pad.astype(ml_dtypes.bfloat16)

    # qw fp8 image [128, (c, h, o)] = qw8[h*128+p, c*512+o]
    qw_img = np.ascontiguousarray(
        qw8.reshape(2, 128, OT, 512).transpose(1, 2, 0, 3).reshape(128, -1))

    in_maps_a = []
    for c in range(N_CORES):
        rows = slice(c * S, (c + 1) * S)
        # kwt [p, (m, h, i)] = kw8[c*640 + m*128 + i, h*128 + p]
        kwt = (kw8[rows].reshape(MT, 128, 2, 128).transpose(3, 0, 2, 1)
               .reshape(128, -1))
        # kw [p, (m, h)] = kwb[c*640 + m*128 + p, h]
        kwc = (kwb[rows].reshape(MT, 128, H_DIM).transpose(1, 0, 2)
               .reshape(128, -1))
        # vv [p, (m, bd)] = v_pad[c*640 + m*128 + p, bd]
        vvc = (vvb[rows].reshape(MT, 128, BD).transpose(1, 0, 2)
               .reshape(128, -1))
        in_maps_a.append({
            "q8": np.ascontiguousarray(np.concatenate([kwt, qw_img], axis=1)),
            "b16": np.ascontiguousarray(np.concatenate([vvc, kwc], axis=1)),
        })

    res_a, a_ns = _run_spmd(nc_a, in_maps_a, profile)
    # host gather: sum the 8 partial 16*T contributions [128, 384]
    tsum = np.sum([r["tv"] for r in res_a], axis=0,
                  dtype=np.float32).astype(ml_dtypes.bfloat16)

    in_maps_b = [{
        # [p, (jb, h, j)] = qwb[h*128 + p, c*640 + jb*128 + j]
        "qwy": np.ascontiguousarray(
            qwb[:, c * S:(c + 1) * S].reshape(2, 128, MT, 128)
            .transpose(1, 2, 0, 3).reshape(128, -1)),
        "tsum": tsum,
    } for c in range(N_CORES)]

    res_b, b_ns = _run_spmd(nc_b, in_maps_b, profile)

    # y_d [p, (jb, bd)] per core -> y_full[c*640 + jb*128 + p, bd]
    y_full = np.empty((N_PAD, BD), np.float32)
    for c in range(N_CORES):
        yc = (res_b[c]["y"].astype(np.float32)
              .reshape(128, MT, BD).transpose(1, 0, 2))
        y_full[c * S:(c + 1) * S] = yc.reshape(S, BD)
    y = np.ascontiguousarray(
        y_full[:N].reshape(N, B, 3).transpose(1, 0, 2)).astype(np.float32)

    LAST_PHASE_A_NS = a_ns
    LAST_PHASE_B_NS = b_ns
    LAST_HW_EXEC_NS = (a_ns or 0) + (b_ns or 0) if profile else None
    return y
